# revision 2
# baseline (speedup 1.0000x reference)
"""Trainium2 Bass kernel for nn_IntraAtt (gnn_message_passing), 8 NeuronCores.

Pipeline (i=0 case of the reference):
  xf = x[mapper]; f = segmean(xf, batch); r = relu(f@W_u + b_u)
  y[v] = x[v] + (1/c_v) * sum_{j:mapper[j]=v} r[batch[j]]   (c_v = count)
  tw = y @ W_gcn
  out[u] = dinv[u] * sum_{msgs e->u} tw[mapper[src_e]] * dinv[src_e]  + b_gcn
  fx = segmean(out, batch)
Executed as 4 SPMD launches over 8 cores with host-side index prep and
re-sharding between launches.  All segment sums are PE matmuls against
on-chip-built selection matrices (is_equal of iota vs per-row local seg id).
"""
import sys
import numpy as np

sys.path.insert(0, "/opt/trn_rl_repo")

import concourse.bass as bass
import concourse.bacc as bacc
import concourse.mybir as mybir
import concourse.tile as tile

NC_ = 8
N_NODES = 150000
N_FRAG = 300000
N_FRAGS = 30000
D = 128
P = 128
FPC = N_FRAGS // NC_          # 3750 fragments per core
NBA = FPC // P + (1 if FPC % P else 0)   # 30 blocks (A and E)
NPC = N_NODES // NC_          # 18750 nodes per core
NBB = NPC // P + (1 if NPC % P else 0)   # 147 blocks (B)

F32 = mybir.dt.float32
I32 = mybir.dt.int32


# ---------------------------------------------------------------- host prep
def _blocked_streams(seg_per_core, payloads_per_core, n_blocks, fills):
    """Pack sorted per-core segment streams into fixed [128, T] tile columns.

    Each block of 128 local segments gets the same tile count on every core
    (max over cores, >=1) so the SPMD graph is identical.  Padded rows get
    seg=-1 (selection matrix kills them) and payload=fills[k]."""
    ncs = len(seg_per_core)
    bounds = np.arange(n_blocks + 1) * P
    los = np.stack([np.searchsorted(s, bounds[:-1]) for s in seg_per_core])
    his = np.stack([np.searchsorted(s, bounds[1:]) for s in seg_per_core])
    tpb = np.maximum(1, -(-(his - los) // P)).max(axis=0)  # tiles per block
    starts = np.concatenate([[0], np.cumsum(tpb)])
    T = int(starts[-1])
    seg_arr = np.full((ncs, T, P), -1.0, np.float32)
    pay = {k: np.full((ncs, T, P), fills[k], dtype=v[k].dtype)
           for k, v in [(kk, payloads_per_core[0]) for kk in payloads_per_core[0]]}
    for c in range(ncs):
        sl = seg_per_core[c]
        for b in range(n_blocks):
            lo, hi = los[c][b], his[c][b]
            nt = -(-(hi - lo) // P)
            for t in range(nt):
                s = lo + t * P
                e = min(s + P, hi)
                m = e - s
                seg_arr[c, starts[b] + t, :m] = sl[s:e] - b * P
                for k in pay:
                    pay[k][c, starts[b] + t, :m] = payloads_per_core[c][k][s:e]
    res = {"seg": [np.ascontiguousarray(seg_arr[c].T) for c in range(ncs)]}
    for k in pay:
        res[k] = [np.ascontiguousarray(pay[k][c].T) for c in range(ncs)]
    return res, tpb.tolist(), T


def _seg_inner(nc, tc, sbuf, psum, table, idx_t, seg_t, iota_t, tpb, t_start,
               block, nrm_t=None):
    """Emit gather + selection + accumulate matmuls for one block.
    Returns the PSUM tile holding [128 segs, 128 feat] sums."""
    ps = psum.tile([P, P], F32, tag="ps_acc")
    nt = tpb[block]
    for t in range(nt):
        tt = t_start + t
        g = sbuf.tile([P, P], F32, tag="g")
        nc.gpsimd.indirect_dma_start(
            out=g[:], out_offset=None, in_=table[:],
            in_offset=bass.IndirectOffsetOnAxis(ap=idx_t[:, tt:tt + 1], axis=0))
        if nrm_t is not None:
            nc.vector.tensor_scalar_mul(g[:], g[:], nrm_t[:, tt:tt + 1])
        s = sbuf.tile([P, P], F32, tag="s")
        nc.vector.tensor_tensor(
            out=s[:], in0=iota_t[:], in1=seg_t[:, tt:tt + 1].to_broadcast([P, P]),
            op=mybir.AluOpType.is_equal)
        nc.tensor.matmul(ps[:], lhsT=s[:], rhs=g[:], start=(t == 0),
                         stop=(t == nt - 1))
    return ps


def _build_A(tpb, T):
    """f = segmean(x[aidx], aseg); r = relu(f @ Wu + bu) -> r_c [NBA*128, 128]"""
    nc = bacc.Bacc("TRN2", target_bir_lowering=False, debug=False,
                   num_devices=NC_)
    xg = nc.declare_dram_parameter("xg", [N_NODES, D], F32, isOutput=False)
    idx = nc.declare_dram_parameter("idx", [P, T], I32, isOutput=False)
    seg = nc.declare_dram_parameter("seg", [P, T], F32, isOutput=False)
    invc = nc.declare_dram_parameter("invc", [P, NBA], F32, isOutput=False)
    iota = nc.declare_dram_parameter("iota", [P, P], F32, isOutput=False)
    ident = nc.declare_dram_parameter("ident", [P, P], F32, isOutput=False)
    wu = nc.declare_dram_parameter("wu", [P, P], F32, isOutput=False)
    bub = nc.declare_dram_parameter("bub", [P, P], F32, isOutput=False)
    r_c = nc.declare_dram_parameter("r_c", [NBA * P, D], F32, isOutput=True)

    with tile.TileContext(nc) as tc:
        with (tc.tile_pool(name="cst", bufs=1) as cst,
              tc.tile_pool(name="sbuf", bufs=6) as sbuf,
              tc.tile_pool(name="psum", bufs=2, space="PSUM") as psum):
            idx_t = cst.tile([P, T], I32)
            nc.sync.dma_start(out=idx_t[:], in_=idx[:])
            seg_t = cst.tile([P, T], F32)
            nc.sync.dma_start(out=seg_t[:], in_=seg[:])
            invc_t = cst.tile([P, NBA], F32)
            nc.sync.dma_start(out=invc_t[:], in_=invc[:])
            iota_t = cst.tile([P, P], F32)
            nc.sync.dma_start(out=iota_t[:], in_=iota[:])
            ident_t = cst.tile([P, P], F32)
            nc.sync.dma_start(out=ident_t[:], in_=ident[:])
            wu_t = cst.tile([P, P], F32)
            nc.sync.dma_start(out=wu_t[:], in_=wu[:])
            bub_t = cst.tile([P, P], F32)
            nc.sync.dma_start(out=bub_t[:], in_=bub[:])
            t0 = 0
            for b in range(NBA):
                ps = _seg_inner(nc, tc, sbuf, psum, xg, idx_t, seg_t, iota_t,
                                tpb, t0, b)
                t0 += tpb[b]
                f_sb = sbuf.tile([P, P], F32, tag="f")
                nc.vector.tensor_scalar_mul(f_sb[:], ps[:], invc_t[:, b:b + 1])
                ft_ps = psum.tile([P, P], F32, tag="tr")
                nc.tensor.transpose(out=ft_ps[:], in_=f_sb[:], identity=ident_t[:])
                ft_sb = sbuf.tile([P, P], F32, tag="ft")
                nc.scalar.copy(out=ft_sb[:], in_=ft_ps[:])
                r_ps = psum.tile([P, P], F32, tag="rps")
                nc.tensor.matmul(r_ps[:], lhsT=ft_sb[:], rhs=wu_t[:],
                                 start=True, stop=True)
                r_sb = sbuf.tile([P, P], F32, tag="r")
                nc.vector.tensor_add(out=r_sb[:], in0=r_ps[:], in1=bub_t[:])
                nc.scalar.activation(out=r_sb[:], in_=r_sb[:],
                                     func=mybir.ActivationFunctionType.Relu)
                nc.sync.dma_start(out=r_c[b * P:(b + 1) * P, :], in_=r_sb[:])
    nc.compile()
    return nc


def _build_B(tpb, T):
    """z = segsum(r[bidx], bseg); tw = (x_slice + z*invc) @ Wgcn"""
    nc = bacc.Bacc("TRN2", target_bir_lowering=False, debug=False,
                   num_devices=NC_)
    rf = nc.declare_dram_parameter("rf", [N_FRAGS, D], F32, isOutput=False)
    idx = nc.declare_dram_parameter("idx", [P, T], I32, isOutput=False)
    seg = nc.declare_dram_parameter("seg", [P, T], F32, isOutput=False)
    invc = nc.declare_dram_parameter("invc", [P, NBB], F32, isOutput=False)
    xsl = nc.declare_dram_parameter("xsl", [NBB * P, D], F32, isOutput=False)
    iota = nc.declare_dram_parameter("iota", [P, P], F32, isOutput=False)
    ident = nc.declare_dram_parameter("ident", [P, P], F32, isOutput=False)
    wg = nc.declare_dram_parameter("wg", [P, P], F32, isOutput=False)
    tw_c = nc.declare_dram_parameter("tw_c", [NBB * P, D], F32, isOutput=True)

    with tile.TileContext(nc) as tc:
        with (tc.tile_pool(name="cst", bufs=1) as cst,
              tc.tile_pool(name="sbuf", bufs=6) as sbuf,
              tc.tile_pool(name="psum", bufs=2, space="PSUM") as psum):
            idx_t = cst.tile([P, T], I32)
            nc.sync.dma_start(out=idx_t[:], in_=idx[:])
            seg_t = cst.tile([P, T], F32)
            nc.sync.dma_start(out=seg_t[:], in_=seg[:])
            invc_t = cst.tile([P, NBB], F32)
            nc.sync.dma_start(out=invc_t[:], in_=invc[:])
            iota_t = cst.tile([P, P], F32)
            nc.sync.dma_start(out=iota_t[:], in_=iota[:])
            ident_t = cst.tile([P, P], F32)
            nc.sync.dma_start(out=ident_t[:], in_=ident[:])
            wg_t = cst.tile([P, P], F32)
            nc.sync.dma_start(out=wg_t[:], in_=wg[:])
            t0 = 0
            for b in range(NBB):
                ps = _seg_inner(nc, tc, sbuf, psum, rf, idx_t, seg_t, iota_t,
                                tpb, t0, b)
                t0 += tpb[b]
                z_sb = sbuf.tile([P, P], F32, tag="z")
                nc.vector.tensor_scalar_mul(z_sb[:], ps[:], invc_t[:, b:b + 1])
                xb = sbuf.tile([P, P], F32, tag="xb")
                nc.sync.dma_start(out=xb[:], in_=xsl[b * P:(b + 1) * P, :])
                nc.vector.tensor_add(out=z_sb[:], in0=z_sb[:], in1=xb[:])
                tt_ps = psum.tile([P, P], F32, tag="tr")
                nc.tensor.transpose(out=tt_ps[:], in_=z_sb[:], identity=ident_t[:])
                tt_sb = sbuf.tile([P, P], F32, tag="tt")
                nc.scalar.copy(out=tt_sb[:], in_=tt_ps[:])
                tw_ps = psum.tile([P, P], F32, tag="twps")
                nc.tensor.matmul(tw_ps[:], lhsT=tt_sb[:], rhs=wg_t[:],
                                 start=True, stop=True)
                tw_sb = sbuf.tile([P, P], F32, tag="tw")
                nc.scalar.copy(out=tw_sb[:], in_=tw_ps[:])
                nc.sync.dma_start(out=tw_c[b * P:(b + 1) * P, :], in_=tw_sb[:])
    nc.compile()
    return nc


def _build_D(tpb, T, nbd):
    """out[u] = dinv[u]*segsum(tw[eidx]*enrm, eseg) + b_gcn"""
    nc = bacc.Bacc("TRN2", target_bir_lowering=False, debug=False,
                   num_devices=NC_)
    twf = nc.declare_dram_parameter("twf", [N_NODES, D], F32, isOutput=False)
    idx = nc.declare_dram_parameter("idx", [P, T], I32, isOutput=False)
    seg = nc.declare_dram_parameter("seg", [P, T], F32, isOutput=False)
    nrm = nc.declare_dram_parameter("nrm", [P, T], F32, isOutput=False)
    dcol = nc.declare_dram_parameter("dcol", [P, nbd], F32, isOutput=False)
    iota = nc.declare_dram_parameter("iota", [P, P], F32, isOutput=False)
    bgb = nc.declare_dram_parameter("bgb", [P, P], F32, isOutput=False)
    out_c = nc.declare_dram_parameter("out_c", [nbd * P, D], F32, isOutput=True)

    with tile.TileContext(nc) as tc:
        with (tc.tile_pool(name="cst", bufs=1) as cst,
              tc.tile_pool(name="sbuf", bufs=6) as sbuf,
              tc.tile_pool(name="psum", bufs=2, space="PSUM") as psum):
            idx_t = cst.tile([P, T], I32)
            nc.sync.dma_start(out=idx_t[:], in_=idx[:])
            seg_t = cst.tile([P, T], F32)
            nc.sync.dma_start(out=seg_t[:], in_=seg[:])
            nrm_t = cst.tile([P, T], F32)
            nc.sync.dma_start(out=nrm_t[:], in_=nrm[:])
            dcol_t = cst.tile([P, nbd], F32)
            nc.sync.dma_start(out=dcol_t[:], in_=dcol[:])
            iota_t = cst.tile([P, P], F32)
            nc.sync.dma_start(out=iota_t[:], in_=iota[:])
            bgb_t = cst.tile([P, P], F32)
            nc.sync.dma_start(out=bgb_t[:], in_=bgb[:])
            t0 = 0
            for b in range(nbd):
                ps = _seg_inner(nc, tc, sbuf, psum, twf, idx_t, seg_t, iota_t,
                                tpb, t0, b, nrm_t=nrm_t)
                t0 += tpb[b]
                o_sb = sbuf.tile([P, P], F32, tag="o")
                nc.vector.tensor_scalar_mul(o_sb[:], ps[:], dcol_t[:, b:b + 1])
                nc.vector.tensor_add(out=o_sb[:], in0=o_sb[:], in1=bgb_t[:])
                nc.sync.dma_start(out=out_c[b * P:(b + 1) * P, :], in_=o_sb[:])
    nc.compile()
    return nc


def _build_E(tpb, T):
    """fx = segmean(e_in rows, dseg) over fragments (direct sequential loads)"""
    nc = bacc.Bacc("TRN2", target_bir_lowering=False, debug=False,
                   num_devices=NC_)
    ein = nc.declare_dram_parameter("ein", [T * P, D], F32, isOutput=False)
    seg = nc.declare_dram_parameter("seg", [P, T], F32, isOutput=False)
    invc = nc.declare_dram_parameter("invc", [P, NBA], F32, isOutput=False)
    iota = nc.declare_dram_parameter("iota", [P, P], F32, isOutput=False)
    fx_c = nc.declare_dram_parameter("fx_c", [NBA * P, D], F32, isOutput=True)

    with tile.TileContext(nc) as tc:
        with (tc.tile_pool(name="cst", bufs=1) as cst,
              tc.tile_pool(name="sbuf", bufs=6) as sbuf,
              tc.tile_pool(name="psum", bufs=2, space="PSUM") as psum):
            seg_t = cst.tile([P, T], F32)
            nc.sync.dma_start(out=seg_t[:], in_=seg[:])
            invc_t = cst.tile([P, NBA], F32)
            nc.sync.dma_start(out=invc_t[:], in_=invc[:])
            iota_t = cst.tile([P, P], F32)
            nc.sync.dma_start(out=iota_t[:], in_=iota[:])
            t0 = 0
            for b in range(NBA):
                ps = psum.tile([P, P], F32, tag="ps_acc")
                nt = tpb[b]
                for t in range(nt):
                    tt = t0 + t
                    g = sbuf.tile([P, P], F32, tag="g")
                    nc.sync.dma_start(out=g[:], in_=ein[tt * P:(tt + 1) * P, :])
                    s = sbuf.tile([P, P], F32, tag="s")
                    nc.vector.tensor_tensor(
                        out=s[:], in0=iota_t[:],
                        in1=seg_t[:, tt:tt + 1].to_broadcast([P, P]),
                        op=mybir.AluOpType.is_equal)
                    nc.tensor.matmul(ps[:], lhsT=s[:], rhs=g[:],
                                     start=(t == 0), stop=(t == nt - 1))
                t0 += nt
                fx_sb = sbuf.tile([P, P], F32, tag="fx")
                nc.vector.tensor_scalar_mul(fx_sb[:], ps[:], invc_t[:, b:b + 1])
                nc.sync.dma_start(out=fx_c[b * P:(b + 1) * P, :], in_=fx_sb[:])
    nc.compile()
    return nc


# ---------------------------------------------------------------- runner
def _make_runner(nc, n_cores=NC_):
    import jax
    from jax.sharding import Mesh, PartitionSpec
    from jax.experimental.shard_map import shard_map
    from concourse import bass2jax
    bass2jax.install_neuronx_cc_hook()
    partition_name = (nc.partition_id_tensor.name
                      if nc.partition_id_tensor else None)
    in_names, out_names, out_avals, zero_outs = [], [], [], []
    for alloc in nc.m.functions[0].allocations:
        if not isinstance(alloc, mybir.MemoryLocationSet):
            continue
        name = alloc.memorylocations[0].name
        if alloc.kind == "ExternalInput":
            if name != partition_name:
                in_names.append(name)
        elif alloc.kind == "ExternalOutput":
            shape = tuple(alloc.tensor_shape)
            dtype = mybir.dt.np(alloc.dtype)
            out_names.append(name)
            out_avals.append(jax.core.ShapedArray(shape, dtype))
            zero_outs.append(np.zeros(shape, dtype))
    n_params = len(in_names)
    n_outs = len(out_avals)
    bind_in_names = in_names + out_names + (
        [partition_name] if partition_name else [])

    def _body(*args):
        operands = list(args)
        if partition_name is not None:
            operands.append(bass2jax.partition_id_tensor())
        res = bass2jax._bass_exec_p.bind(
            *operands, out_avals=tuple(out_avals),
            in_names=tuple(bind_in_names), out_names=tuple(out_names),
            lowering_input_output_aliases=(), sim_require_finite=True,
            sim_require_nnan=True, nc=nc)
        return tuple(res)

    devices = jax.devices()[:n_cores]
    mesh = Mesh(np.asarray(devices), ("core",))
    in_specs = (PartitionSpec("core"),) * (n_params + n_outs)
    out_specs = (PartitionSpec("core"),) * n_outs
    sharded = jax.jit(shard_map(_body, mesh=mesh, in_specs=in_specs,
                                out_specs=out_specs, check_rep=False))

    def run(in_maps):
        import time as _time
        import jax as _jax
        concat_in = [np.concatenate([np.asarray(in_maps[c][k])
                                     for c in range(n_cores)], axis=0)
                     for k in in_names]
        concat_zeros = [np.zeros((n_cores * z.shape[0], *z.shape[1:]), z.dtype)
                        for z in zero_outs]
        args = [ _jax.device_put(a) for a in concat_in + concat_zeros ]
        for a in args:
            a.block_until_ready()
        t0 = _time.perf_counter()
        outs = sharded(*args)
        _jax.block_until_ready(outs)
        wall = _time.perf_counter() - t0
        results = [
            {name: np.asarray(outs[i]).reshape(n_cores, *out_avals[i].shape)[c]
             for i, name in enumerate(out_names)}
            for c in range(n_cores)]
        return results, wall
    return run


_CACHE = {}


def kernel(x, fragments_nodes_mapper, fragments_batch, edge_index,
           W_u, b_u, W_gcn, b_gcn, i):
    x = np.asarray(x, np.float32)
    mapper = np.asarray(fragments_nodes_mapper, np.int64)
    batch = np.asarray(fragments_batch, np.int64)
    edges = np.asarray(edge_index, np.int64)
    W_u = np.asarray(W_u, np.float32)
    b_u = np.asarray(b_u, np.float32)
    W_gcn = np.asarray(W_gcn, np.float32)
    b_gcn = np.asarray(b_gcn, np.float32)

    iota = np.tile(np.arange(P, dtype=np.float32), (P, 1))
    ident = np.eye(P, dtype=np.float32)
    bub = np.tile(b_u[None, :], (P, 1)).astype(np.float32)
    bgb = np.tile(b_gcn[None, :], (P, 1)).astype(np.float32)

    # ---- A prep: fragment-aligned row split
    fs = np.searchsorted(batch, np.arange(NC_ + 1) * FPC)
    cnt_frag = np.bincount(batch, minlength=N_FRAGS).astype(np.float32)
    invc_frag = (1.0 / np.maximum(cnt_frag, 1.0)).astype(np.float32)
    a_seg, a_idx = [], []
    for c in range(NC_):
        rows = np.arange(fs[c], fs[c + 1])
        a_seg.append((batch[rows] - c * FPC).astype(np.int64))
        a_idx.append(mapper[rows].astype(np.int32))
    sa, tpb_a, TA = _blocked_streams(
        a_seg, [{"idx": a_idx[c]} for c in range(NC_)], NBA, {"idx": 0})
    invc_a = [invc_frag[c * FPC:(c + 1) * FPC] for c in range(NC_)]
    invc_a = [np.concatenate([v, np.zeros(NBA * P - FPC, np.float32)])
              .reshape(NBA, P).T.copy() for v in invc_a]

    # ---- B prep: shard nodes
    cnt_node = np.bincount(mapper, minlength=N_NODES).astype(np.float32)
    invc_node = (1.0 / np.maximum(cnt_node, 1.0)).astype(np.float32)
    owner = mapper // NPC
    order = np.argsort(mapper, kind="stable")
    b_seg, b_idx = [], []
    for c in range(NC_):
        j = order[(owner[order] == c)]
        b_seg.append((mapper[j] - c * NPC).astype(np.int64))
        b_idx.append(batch[j].astype(np.int32))
    sb, tpb_b, TB = _blocked_streams(
        b_seg, [{"idx": b_idx[c]} for c in range(NC_)], NBB, {"idx": 0})
    padb = NBB * P - NPC
    invc_b = [np.concatenate([invc_node[c * NPC:(c + 1) * NPC],
                              np.zeros(padb, np.float32)])
              .reshape(NBB, P).T.copy() for c in range(NC_)]
    xsl = [np.concatenate([x[c * NPC:(c + 1) * NPC],
                           np.zeros((padb, D), np.float32)]) for c in range(NC_)]

    # ---- D prep: edges + self loops, sharded by dest (fragment-aligned)
    row, col = edges[0], edges[1]
    deg = (np.bincount(col, minlength=N_FRAG) + 1).astype(np.float32)
    dinv = (1.0 / np.sqrt(deg)).astype(np.float32)
    msrc = np.concatenate([row, np.arange(N_FRAG)])
    mdst = np.concatenate([col, np.arange(N_FRAG)])
    dorder = np.argsort(mdst, kind="stable")
    msrc = msrc[dorder]
    mdst = mdst[dorder]
    csplit = np.searchsorted(mdst, fs)
    nloc = (fs[1:] - fs[:-1]).astype(int)
    nbd = int(-(-nloc.max() // P))
    d_seg, d_idx, d_nrm = [], [], []
    for c in range(NC_):
        sl = slice(csplit[c], csplit[c + 1])
        d_seg.append((mdst[sl] - fs[c]).astype(np.int64))
        d_idx.append(mapper[msrc[sl]].astype(np.int32))
        d_nrm.append(dinv[msrc[sl]].astype(np.float32))
    sd, tpb_d, TD = _blocked_streams(
        d_seg, [{"idx": d_idx[c], "nrm": d_nrm[c]} for c in range(NC_)],
        nbd, {"idx": 0, "nrm": 0.0})
    dcol = []
    for c in range(NC_):
        v = np.zeros(nbd * P, np.float32)
        v[:nloc[c]] = dinv[fs[c]:fs[c + 1]]
        dcol.append(v.reshape(nbd, P).T.copy())

    # ---- build/compile graphs (cached)
    key = ("v1", TA, TB, TD, nbd)
    if key not in _CACHE:
        runs = {}
        runs["A"] = _make_runner(_build_A(tpb_a, TA))
        runs["B"] = _make_runner(_build_B(tpb_b, TB))
        runs["D"] = _make_runner(_build_D(tpb_d, TD, nbd))
        _CACHE[key] = runs
    runs = _CACHE[key]

    walls = {}
    # ---- launch A
    insA = [{"xg": x, "idx": sa["idx"][c], "seg": sa["seg"][c],
             "invc": invc_a[c], "iota": iota, "ident": ident,
             "wu": W_u, "bub": bub} for c in range(NC_)]
    resA, walls["A"] = runs["A"](insA)
    r_full = np.concatenate([resA[c]["r_c"][:FPC] for c in range(NC_)])

    # ---- launch B
    insB = [{"rf": r_full, "idx": sb["idx"][c], "seg": sb["seg"][c],
             "invc": invc_b[c], "xsl": xsl[c], "iota": iota, "ident": ident,
             "wg": W_gcn} for c in range(NC_)]
    resB, walls["B"] = runs["B"](insB)
    tw_full = np.concatenate([resB[c]["tw_c"][:NPC] for c in range(NC_)])

    # ---- launch D
    insD = [{"twf": tw_full, "idx": sd["idx"][c], "seg": sd["seg"][c],
             "nrm": sd["nrm"][c], "dcol": dcol[c], "iota": iota, "bgb": bgb}
            for c in range(NC_)]
    resD, walls["D"] = runs["D"](insD)
    out_full = np.concatenate([resD[c]["out_c"][:nloc[c]] for c in range(NC_)])

    # ---- launch E prep: repack out rows into fragment-aligned blocks
    e_seg = [(batch[np.arange(fs[c], fs[c + 1])] - c * FPC).astype(np.int64)
             for c in range(NC_)]
    se, tpb_e, TE = _blocked_streams(
        e_seg, [{"pos": np.arange(fs[c], fs[c + 1]).astype(np.int64)}
                for c in range(NC_)], NBA, {"pos": -1})
    keyE = ("vE", TE)
    if keyE not in _CACHE:
        _CACHE[keyE] = _make_runner(_build_E(tpb_e, TE))
    runE = _CACHE[keyE]
    insE = []
    for c in range(NC_):
        pos = se["pos"][c].T.reshape(-1)   # [T*P] row positions, -1 = pad
        ein = np.zeros((TE * P, D), np.float32)
        valid = pos >= 0
        ein[valid] = out_full[pos[valid]]
        insE.append({"ein": ein, "seg": se["seg"][c], "invc": invc_a[c],
                     "iota": iota})
    resE, walls["E"] = runE(insE)
    fx = np.concatenate([resE[c]["fx_c"][:FPC] for c in range(NC_)])

    kernel._last_walls = walls
    return (fx, out_full)
